# revision 15
# baseline (speedup 1.0000x reference)
"""BiLSTM-CRF loss kernel for 8 Trainium2 NeuronCores.

Strategy (data-parallel over batch, 16 sequences/core):
  - Embedding gather via indirect DMA, PE-transpose to feature-major X.T.
  - Fwd+bwd LSTM interleaved in one 256-step scan. Recurrent + input
    projections + biases all done as PSUM-accumulated matmuls using
    4-way tensor-engine column tiling (M=16 per col group).
  - Gates batch-major in partition stripes; sigmoid/tanh on ScalarE,
    cell update on VectorE; h transposed back via PE each step, stored
    to a feature-major history.
  - feats = h_hist @ W_h2t.T as big matmuls; CRF forward algorithm run
    in probability space (exp-transition matmul per step on PE) with
    periodic rescaling; gold score via one-hot compares.
Each core returns a partial loss; host sums the 8 partials.
"""
import sys

sys.path.insert(0, "/opt/trn_rl_repo")

import numpy as np
from contextlib import ExitStack

import concourse.bass as bass
import concourse.bacc as bacc
import concourse.tile as tile
from concourse import mybir
from concourse.masks import make_identity

F32 = mybir.dt.float32
I32 = mybir.dt.int32
AF = mybir.ActivationFunctionType
ALU = mybir.AluOpType

V, E, HID, T_FULL, B_FULL, NT = 50000, 256, 512, 256, 128, 10
H2 = HID // 2
START, STOP = 8, 9
NCORES = 8
BC = B_FULL // NCORES  # 16 sequences per core
RESCALE = 6


def build_nc(T=T_FULL):
    """Build the per-core Bass program (same program for all 8 cores)."""
    nc = bacc.Bacc("TRN2", target_bir_lowering=False)

    ntok = T * BC                      # tokens per core
    ngather = ntok // 128              # indirect-DMA tiles of 128 rows
    tpg = min(32, T)                   # timesteps per feats group
    ngroups = T // tpg                 # feats groups (N = tpg*16 <= 512)
    nresc = T // RESCALE               # CRF rescale events
    npairs = (T + 1) * BC              # gold-score transition pairs
    pair_cols = (npairs + 255) // 128  # [128, pair_cols] padded layout

    # ---- external inputs ----
    sent_idx = nc.dram_tensor("sent_idx", [128, ngather], I32, kind="ExternalInput")
    tags_tm = nc.dram_tensor("tags_tm", [1, ntok], F32, kind="ExternalInput")
    pair_next = nc.dram_tensor("pair_next", [128, pair_cols], F32, kind="ExternalInput")
    pair_prev = nc.dram_tensor("pair_prev", [128, pair_cols], F32, kind="ExternalInput")
    embed_d = nc.dram_tensor("embed", [V, E], F32, kind="ExternalInput")
    wstream_f = nc.dram_tensor("wstream_f", [128, 4, 1024], F32, kind="ExternalInput")
    wstream_b = nc.dram_tensor("wstream_b", [128, 4, 1024], F32, kind="ExternalInput")
    bias_fb = nc.dram_tensor("bias_fb", [1, 2048], F32, kind="ExternalInput")
    wh2t_d = nc.dram_tensor("wh2t", [128, 4, 10], F32, kind="ExternalInput")
    b2t_d = nc.dram_tensor("b2t", [10, 1], F32, kind="ExternalInput")
    b2t_row_d = nc.dram_tensor("b2t_row", [1, 10], F32, kind="ExternalInput")
    exptt_d = nc.dram_tensor("exptt", [10, 10], F32, kind="ExternalInput")
    expstop_d = nc.dram_tensor("expstop", [10, 1], F32, kind="ExternalInput")
    transflat_d = nc.dram_tensor("transflat", [1, 100], F32, kind="ExternalInput")
    loss_d = nc.dram_tensor("loss", [1, 1], F32, kind="ExternalOutput")

    with tile.TileContext(nc) as tc, ExitStack() as ctx:
        const = ctx.enter_context(tc.tile_pool(name="const", bufs=1))
        big = ctx.enter_context(tc.tile_pool(name="big", bufs=1))
        act = ctx.enter_context(tc.tile_pool(name="act", bufs=2))
        upd = ctx.enter_context(tc.tile_pool(name="upd", bufs=2))

        # ---- constants ----
        ident = const.tile([128, 128], F32)
        make_identity(nc, ident[:])
        ones1x16 = const.tile([1, 16], F32)
        nc.vector.memset(ones1x16[:], 1.0)
        ones1x32 = const.tile([1, 32], F32)
        nc.vector.memset(ones1x32[:], 1.0)
        ones10x1 = const.tile([10, 1], F32)
        nc.vector.memset(ones10x1[:], 1.0)
        ones1x10 = const.tile([1, 10], F32)
        nc.vector.memset(ones1x10[:], 1.0)
        ones128 = const.tile([128, 1], F32)
        nc.vector.memset(ones128[:], 1.0)
        ones1x512 = const.tile([1, 512], F32)
        nc.vector.memset(ones1x512[:], 1.0)

        wtile_f = const.tile([128, 4, 1024], F32)
        nc.sync.dma_start(wtile_f[:], wstream_f[:])
        wtile_b = const.tile([128, 4, 1024], F32)
        nc.sync.dma_start(wtile_b[:], wstream_b[:])
        bias_t = const.tile([1, 2048], F32)
        nc.sync.dma_start(bias_t[:], bias_fb[:])
        wh2t_t = const.tile([128, 4, 10], F32)
        nc.sync.dma_start(wh2t_t[:], wh2t_d[:])
        b2t_t = const.tile([10, 1], F32)
        nc.sync.dma_start(b2t_t[:], b2t_d[:])
        b2t_row_t = const.tile([1, 10], F32)
        nc.sync.dma_start(b2t_row_t[:], b2t_row_d[:])
        exptt_t = const.tile([10, 10], F32)
        nc.sync.dma_start(exptt_t[:], exptt_d[:])
        expstop_t = const.tile([10, 1], F32)
        nc.sync.dma_start(expstop_t[:], expstop_d[:])
        sidx = const.tile([128, ngather], I32)
        nc.sync.dma_start(sidx[:], sent_idx[:])

        # ---- phase 0: gather embeddings, transpose to X.T ----
        ef_cm = tc.tile_pool(name="ef", bufs=1)
        ef_pool = ef_cm.__enter__()
        hist_cm = tc.tile_pool(name="hist", bufs=1)
        hist_pool = hist_cm.__enter__()
        xt_cm = tc.tile_pool(name="xt", bufs=1)
        xt_pool = xt_cm.__enter__()
        xT0 = xt_pool.tile([128, ntok], F32)
        xT1 = xt_pool.tile([128, ntok], F32)
        xTs = [xT0, xT1]
        with tc.tile_pool(name="ps0", bufs=3, space="PSUM") as ps0, \
             tc.tile_pool(name="gath", bufs=3) as gath:
            for c in range(ngather):
                g = gath.tile([128, 256], F32, tag="g")
                nc.gpsimd.indirect_dma_start(
                    out=g[:],
                    out_offset=None,
                    in_=embed_d[:],
                    in_offset=bass.IndirectOffsetOnAxis(ap=sidx[:, c : c + 1], axis=0),
                )
                for j in range(2):
                    pt = ps0.tile([128, 128], F32, tag="pt")
                    nc.tensor.transpose(pt[:], g[:, j * 128 : (j + 1) * 128], ident[:])
                    nc.vector.tensor_copy(xTs[j][:, c * 128 : (c + 1) * 128], pt[:])

        # ---- phase 1: interleaved fwd/bwd LSTM scan ----
        # psum gate stripes (rows): 0:16 f[i,f], 32:48 b[i,f],
        #                           64:80 f[g,o], 96:112 b[g,o]
        # hist layout: [128, T, 32]; cols 0:16 = h_f.T, 16:32 = h_b.T
        hist0 = hist_pool.tile([128, T, 32], F32)
        hist1 = hist_pool.tile([128, T, 32], F32)
        hists = [hist0, hist1]

        with tc.tile_pool(name="ps1", bufs=2, space="PSUM") as ps1, \
             tc.tile_pool(name="ps1T", bufs=2, space="PSUM") as ps1T:
            c_prev = upd.tile([48, 256], F32, tag="cstate")
            nc.vector.memset(c_prev[:], 0.0)

            for t in range(T):
                tb = T - 1 - t  # bwd timestep
                pg = ps1.tile([128, 512], F32, tag="gates")
                for j, (wt, toff, half) in enumerate(
                    [(wtile_f, t, 0), (wtile_b, tb, 0),
                     (wtile_f, t, 1), (wtile_b, tb, 1)]
                ):
                    tp = (0, 32 * j)
                    out = pg[32 * j : 32 * j + 16, :]
                    out32 = pg[32 * j : 32 * j + 32, :]
                    nsl = slice(512 * half, 512 * half + 512)
                    dcol = 0 if wt is wtile_f else 1
                    boff = 1024 * dcol + 512 * half
                    mms = [(ones1x32[:], bias_t[:, boff : boff + 512])]
                    for k in range(2):
                        mms.append(
                            (xTs[k][:, toff * 16 : toff * 16 + 16], wt[:, k, nsl])
                        )
                    if t > 0:
                        hprev = t - 1 if wt is wtile_f else tb + 1
                        hoff = 0 if wt is wtile_f else 16
                        for k in range(2):
                            mms.append(
                                (hists[k][:, hprev, hoff : hoff + 16],
                                 wt[:, 2 + k, nsl])
                            )
                    for i, (lhsT, rhs) in enumerate(mms):
                        nc.tensor.matmul(
                            out32 if i == 0 else out, lhsT, rhs,
                            start=(i == 0), stop=(i == len(mms) - 1),
                            tile_position=tp,
                            skip_group_check=True,
                        )
                # activations: sigmoid everywhere, tanh over the g region
                sig = act.tile([112, 512], F32, tag="sig")
                nc.scalar.activation(sig[:], pg[0:112, :], AF.Sigmoid)
                tnh = act.tile([48, 256], F32, tag="tnh")
                nc.scalar.activation(tnh[:], pg[64:112, 0:256], AF.Tanh)
                # cell update (rows 0:16 fwd, 32:48 bwd, 16:32 garbage)
                fc = upd.tile([48, 256], F32, tag="fc")
                nc.vector.tensor_mul(fc[:], sig[0:48, 256:512], c_prev[:])
                ig = upd.tile([48, 256], F32, tag="ig")
                nc.vector.tensor_mul(ig[:], sig[0:48, 0:256], tnh[:])
                c_new = upd.tile([48, 256], F32, tag="cstate")
                nc.vector.tensor_add(c_new[:], fc[:], ig[:])
                # write tanh(c) at base partition 64 so the h-mul's two SBUF
                # inputs share a base partition (HW constraint)
                tc_t = act.tile([112, 256], F32, tag="tanc")
                nc.scalar.activation(tc_t[64:112, :], c_new[:], AF.Tanh)
                h_new = upd.tile([48, 256], F32, tag="h")
                nc.vector.tensor_mul(
                    h_new[:], sig[64:112, 256:512], tc_t[64:112, :]
                )
                c_prev = c_new
                # transpose h -> feature-major, store to history
                pt = ps1T.tile([128, 2, 48], F32, tag="hT")
                for k in range(2):
                    nc.tensor.transpose(
                        pt[:, k, :], h_new[:, k * 128 : (k + 1) * 128],
                        ident[0:48, 0:48],
                    )
                for k in range(2):
                    nc.vector.tensor_copy(hists[k][:, t, 0:16], pt[:, k, 0:16])
                    nc.vector.tensor_copy(hists[k][:, tb, 16:32], pt[:, k, 32:48])

        xt_cm.__exit__(None, None, None)

        # ---- phase 2: feats + exp(feats), gold emit score ----
        expfeat = ef_pool.tile([10, ntok], F32)
        tags10 = ef_pool.tile([10, ntok], F32)
        nc.sync.dma_start(
            tags10[:], bass.AP(tensor=tags_tm, offset=0, ap=[[0, 10], [1, ntok]])
        )
        iota10 = const.tile([10, 1], F32)
        nc.gpsimd.iota(
            iota10[:], pattern=[[0, 1]], base=0, channel_multiplier=1,
            allow_small_or_imprecise_dtypes=True,
        )
        emit_acc = None
        with tc.tile_pool(name="ps2", bufs=2, space="PSUM") as ps2:
            for g in range(ngroups):
                n = tpg * 16
                pf = ps2.tile([10, n], F32, tag="feats")
                nc.tensor.matmul(
                    pf[:], b2t_row_t[:], ones1x512[:, 0:n], start=True, stop=False
                )
                for k in range(4):
                    hk = hists[k % 2]
                    hoff = 0 if k < 2 else 16
                    rhs = hk[:, g * tpg : (g + 1) * tpg, hoff : hoff + 16]
                    nc.tensor.matmul(
                        pf[:], wh2t_t[:, k, :], rhs, start=False, stop=(k == 3)
                    )
                # gold emit partial: sum(onehot * feats)
                oh = act.tile([10, n], F32, tag="oh")
                nc.vector.tensor_tensor(
                    out=oh[:],
                    in0=tags10[:, g * n : (g + 1) * n],
                    in1=iota10[:].to_broadcast([10, n]),
                    op=ALU.is_equal,
                )
                prod = act.tile([10, n], F32, tag="prod")
                nc.vector.tensor_mul(prod[:], oh[:], pf[:])
                part = act.tile([10, 1], F32, tag="emitpart")
                nc.vector.reduce_sum(part[:], prod[:], axis=mybir.AxisListType.X)
                new_acc = const.tile([10, 1], F32, tag=f"emit{g % 2}")
                if emit_acc is None:
                    nc.vector.tensor_copy(new_acc[:], part[:])
                else:
                    nc.vector.tensor_add(new_acc[:], part[:], emit_acc[:])
                emit_acc = new_acc
                # exp(feats + b2t) for the prob-space CRF
                nc.scalar.activation(
                    expfeat[:, g * n : (g + 1) * n], pf[:], AF.Exp
                )

        hist_cm.__exit__(None, None, None)

        # ---- phase 3: prob-space CRF forward scan ----
        slog = big.tile([1, 16, max(nresc, 1)], F32)
        fwdsum = upd.tile([1, 1], F32)
        with tc.tile_pool(name="ps3", bufs=2, space="PSUM") as ps3:
            a0 = upd.tile([10, 1], F32)
            nc.vector.tensor_scalar(
                a0[:], iota10[:], float(START), None, op0=ALU.is_equal
            )
            alpha = upd.tile([10, 16], F32, tag="alpha")
            nc.vector.tensor_copy(alpha[:], a0[:].to_broadcast([10, 16]))
            ri = 0
            for t in range(T):
                pa = ps3.tile([10, 16], F32, tag="alpha_ps")
                nc.tensor.matmul(pa[:], exptt_t[:], alpha[:], start=True, stop=True)
                alpha = upd.tile([10, 16], F32, tag="alpha")
                nc.vector.tensor_mul(
                    alpha[:], pa[:], expfeat[:, t * 16 : (t + 1) * 16]
                )
                if t % RESCALE == RESCALE - 1 and ri < nresc:
                    ps = ps3.tile([1, 16], F32, tag="s_ps")
                    nc.tensor.matmul(
                        ps[:], ones10x1[:], alpha[:], start=True, stop=True
                    )
                    s_sb = upd.tile([1, 16], F32, tag="s_sb")
                    nc.vector.tensor_copy(s_sb[:], ps[:])
                    nc.vector.tensor_copy(slog[:, :, ri], s_sb[:])
                    ps10 = ps3.tile([10, 16], F32, tag="s10_ps")
                    nc.tensor.matmul(
                        ps10[:], ones1x10[:], s_sb[:], start=True, stop=True
                    )
                    rinv = upd.tile([10, 16], F32, tag="rinv")
                    nc.vector.reciprocal(rinv[:], ps10[:])
                    a2 = upd.tile([10, 16], F32, tag="alpha")
                    nc.vector.tensor_mul(a2[:], alpha[:], rinv[:])
                    alpha = a2
                    ri += 1

            # finalize forward score
            az = upd.tile([10, 16], F32)
            nc.vector.tensor_mul(
                az[:], alpha[:], expstop_t[:].to_broadcast([10, 16])
            )
            pz = ps3.tile([1, 16], F32, tag="s_ps")
            nc.tensor.matmul(pz[:], ones10x1[:], az[:], start=True, stop=True)
            lz = upd.tile([1, 16], F32)
            nc.scalar.activation(lz[:], pz[:], AF.Ln)
            fwd = upd.tile([1, 16], F32)
            if nresc > 0:
                lsl = big.tile([1, 16, nresc], F32)
                nc.scalar.activation(
                    lsl[:].rearrange("p a b -> p (a b)"),
                    slog[:, :, 0:nresc].rearrange("p a b -> p (a b)"),
                    AF.Ln,
                )
                sls = upd.tile([1, 16], F32)
                nc.vector.reduce_sum(sls[:], lsl[:], axis=mybir.AxisListType.X)
                nc.vector.tensor_add(fwd[:], lz[:], sls[:])
            else:
                nc.vector.tensor_copy(fwd[:], lz[:])
            nc.vector.reduce_sum(fwdsum[:], fwd[:], axis=mybir.AxisListType.X)

            # ---- phase 4: gold transition score ----
            ef_cm.__exit__(None, None, None)
            gold_pool = ctx.enter_context(tc.tile_pool(name="gold", bufs=1))
            pn = gold_pool.tile([128, pair_cols], F32)
            nc.sync.dma_start(pn[:], pair_next[:])
            pp = gold_pool.tile([128, pair_cols], F32)
            nc.sync.dma_start(pp[:], pair_prev[:])
            idx = gold_pool.tile([128, pair_cols], F32)
            nc.vector.scalar_tensor_tensor(
                out=idx[:], in0=pn[:], scalar=10.0, in1=pp[:],
                op0=ALU.mult, op1=ALU.add,
            )
            iota100 = gold_pool.tile([128, pair_cols, 100], F32)
            nc.gpsimd.iota(
                iota100[:], pattern=[[0, pair_cols], [1, 100]], base=0,
                channel_multiplier=0, allow_small_or_imprecise_dtypes=True,
            )
            oh100 = gold_pool.tile([128, pair_cols, 100], F32)
            nc.vector.tensor_tensor(
                out=oh100[:],
                in0=idx[:, :, None].to_broadcast([128, pair_cols, 100]),
                in1=iota100[:],
                op=ALU.is_equal,
            )
            tf128 = big.tile([128, 100], F32)
            nc.sync.dma_start(
                tf128[:],
                bass.AP(tensor=transflat_d, offset=0, ap=[[0, 128], [1, 100]]),
            )
            tfb = gold_pool.tile([128, pair_cols, 100], F32)
            nc.vector.tensor_copy(
                tfb[:], tf128[:, None, :].to_broadcast([128, pair_cols, 100])
            )
            prod2 = gold_pool.tile([128, pair_cols, 100], F32)
            nc.vector.tensor_mul(prod2[:], oh100[:], tfb[:])
            transsc = upd.tile([128, 1], F32)
            nc.vector.reduce_sum(
                transsc[:], prod2[:], axis=mybir.AxisListType.XY
            )

            # ---- final: loss = sum(fwd) - sum(emit) - sum(trans) ----
            asm = upd.tile([128, 2], F32)
            nc.vector.memset(asm[:], 0.0)
            nc.vector.tensor_copy(asm[:, 0:1], transsc[:])
            nc.vector.tensor_copy(asm[0:10, 1:2], emit_acc[:])
            pfin = ps3.tile([1, 2], F32, tag="s_ps")
            nc.tensor.matmul(pfin[:], ones128[:], asm[:], start=True, stop=True)
            t1 = upd.tile([1, 1], F32)
            nc.vector.tensor_sub(t1[:], fwdsum[:], pfin[:, 0:1])
            loss_sb = upd.tile([1, 1], F32)
            nc.vector.tensor_sub(loss_sb[:], t1[:], pfin[:, 1:2])
            nc.sync.dma_start(loss_d[:], loss_sb[:])

    nc.compile()
    return nc


def _prep_shared(inputs):
    """Host-side weight/constant prep shared by all cores (layout only)."""
    f32 = lambda x: np.ascontiguousarray(np.asarray(x, dtype=np.float32))
    Wih_f, Whh_f = f32(inputs["Wih_f"]), f32(inputs["Whh_f"])
    Wih_b, Whh_b = f32(inputs["Wih_b"]), f32(inputs["Whh_b"])

    def stream(Wih, Whh):
        WihT, WhhT = Wih.T, Whh.T  # [256, 1024]
        chunks = [WihT[0:128], WihT[128:256], WhhT[0:128], WhhT[128:256]]
        return np.ascontiguousarray(np.stack(chunks, axis=1))  # [128, 4, 1024]

    bias_f = f32(inputs["bih_f"]) + f32(inputs["bhh_f"])
    bias_b = f32(inputs["bih_b"]) + f32(inputs["bhh_b"])
    Wt = f32(inputs["W_h2t"]).T  # [512, 10]
    wh2t = np.ascontiguousarray(
        np.stack([Wt[0:128], Wt[128:256], Wt[256:384], Wt[384:512]], axis=1)
    )  # [128, 4, 10]
    trans = f32(inputs["transitions"])
    return dict(
        embed=f32(inputs["embed"]),
        wstream_f=stream(Wih_f, Whh_f),
        wstream_b=stream(Wih_b, Whh_b),
        bias_fb=np.concatenate([bias_f, bias_b])[None, :].astype(np.float32),
        wh2t=wh2t,
        b2t=f32(inputs["b_h2t"]).reshape(10, 1),
        b2t_row=f32(inputs["b_h2t"]).reshape(1, 10),
        exptt=np.ascontiguousarray(np.exp(trans).T.astype(np.float32)),
        expstop=np.exp(trans[STOP, :]).reshape(10, 1).astype(np.float32),
        transflat=trans.reshape(1, 100),
    )


def _prep_core(sent, tags, T=T_FULL):
    """Per-core index layouts (pure reshapes of the input id tensors)."""
    ntok = T * BC
    ngather = ntok // 128
    npairs = (T + 1) * BC
    pair_cols = (npairs + 255) // 128

    toks = np.arange(ntok)
    # token tok = t*BC + b ; gather tile layout [p, c] with tok = c*128 + p
    sent_idx = sent[toks % BC, toks // BC].reshape(ngather, 128).T.astype(np.int32)
    tags_tm = tags[toks % BC, toks // BC].reshape(1, ntok).astype(np.float32)

    nxt = np.concatenate([tags, np.full((BC, 1), STOP, tags.dtype)], axis=1)
    prv = np.concatenate([np.full((BC, 1), START, tags.dtype), tags], axis=1)
    pn = np.full(128 * pair_cols, -1.0, np.float32)
    pv = np.zeros(128 * pair_cols, np.float32)
    pn[:npairs] = nxt.reshape(-1)
    pv[:npairs] = prv.reshape(-1)
    return dict(
        sent_idx=np.ascontiguousarray(sent_idx),
        tags_tm=tags_tm,
        pair_next=pn.reshape(128, pair_cols),
        pair_prev=pv.reshape(128, pair_cols),
    )


_NC_CACHE = {}


def get_nc(T=T_FULL):
    if T not in _NC_CACHE:
        _NC_CACHE[T] = build_nc(T)
    return _NC_CACHE[T]


def kernel(**inputs):
    from concourse.bass_utils import run_bass_kernel_spmd

    sentences = np.asarray(inputs["sentences"])
    tags = np.asarray(inputs["tags"])
    shared = _prep_shared(inputs)
    nc = get_nc()
    in_maps = []
    for k in range(NCORES):
        sl = slice(k * BC, (k + 1) * BC)
        m = dict(shared)
        m.update(_prep_core(sentences[sl], tags[sl]))
        in_maps.append(m)
    res = run_bass_kernel_spmd(nc, in_maps, core_ids=list(range(NCORES)))
    total = np.float32(0.0)
    for r in res.results:
        total += np.float32(r["loss"][0, 0])
    return np.asarray(total, dtype=np.float32)
